# revision 2
# baseline (speedup 1.0000x reference)
"""Single-head causal attention on 8 TRN2 NeuronCores.

Problem: x[8, 2048, 1024] f32; Wq/Wk/Wv[1024, 128]; bq/bk/bv[128].
  q = x@Wq+bq; k = x@Wk+bk; v = x@Wv+bv
  scores[b,t,s] = k[b,t,:].q[b,s,:] / sqrt(128), causal (s<=t), softmax over s
  out = weights @ v   -> [8, 2048, 128] f32

Sharding: data-parallel over batch, one batch element per core. No collectives.

Per-core algorithm (T=2048, D=1024, H=128), matmuls in bf16. Design notes:
  - scores computed TRANSPOSED: S_T[s, t] = qT.T @ kT, so P_T = exp(S_T) is
    directly the stationary operand of out[t, 129] = P_T.T @ v_aug; the ones
    column of v_aug yields the softmax denominator for free.
  - bv is folded into the v rows (v' = v + bv): softmax weights sum to 1, so
    out/denom + bv == (P@(v+bv))/denom.  Kills the epilogue bias add.
  - input DMA triggers cost ~600-700ns of queue time each; they are split
    between the SP and ACT hardware DGE queues, ordered critical-first.
  - ONE set of PSUM pools lives for the whole kernel (no pool handoff
    barriers): proj ring (2 banks) + S ring (3) + v/O accumulator ring (3).
  - rows are processed in DESCENDING index order: S row si needs k cols
    >= 128*si, so high rows only need the last projection chunks; exp on ACT
    (the #2 engine) starts ~16us instead of ~31us and overlaps projections.
  - group 1 interleaves {k3,q3} proj with v rows 15..13 dc-outer so the PE
    tracks the xT DMA arrival rate chunk by chunk.
  - O accumulation chains run si DESCENDING (start=si==tj, stop=si==0): the
    last-exp'd rows (0..3) are needed at the END of each chain; the last O
    tile (t0) is the shortest chain, minimizing the serial tail.
"""

import math

import ml_dtypes
import numpy as np

import concourse.bass as bass
import concourse.mybir as mybir
import concourse.tile as tile
from concourse import bacc
from concourse.bass_utils import run_bass_kernel_spmd

B, T, D, H = 8, 2048, 1024, 128
NT = T // 128          # 16 t/s tiles
ND = D // 128          # 8 contraction chunks
SCALE = 1.0 / math.sqrt(H)

F32 = mybir.dt.float32
BF16 = mybir.dt.bfloat16
AF = mybir.ActivationFunctionType


def build_nc():
    nc = bacc.Bacc(
        "TRN2",
        target_bir_lowering=False,
        debug=False,
        num_devices=8,
    )

    # x[b].T split into 8 d-chunks x 2 column halves: B half = cols 1024:2048
    # (consumed first), A half = cols 0:1024. Each [128, 1024] contiguous.
    xb_d = nc.dram_tensor("xb", [ND, 128, 1024], BF16, kind="ExternalInput")
    xa_d = nc.dram_tensor("xa", [ND, 128, 1024], BF16, kind="ExternalInput")
    w_d = {
        p: nc.dram_tensor(f"w{p}", [128, ND, H], BF16, kind="ExternalInput")
        for p in ("q", "k", "v")
    }
    bias_d = nc.dram_tensor("bias", [H, 2], F32, kind="ExternalInput")
    mask_d = nc.dram_tensor("mask", [128, 128], BF16, kind="ExternalInput")
    bvb_d = nc.dram_tensor("bvb", [128, 128], F32, kind="ExternalInput")
    out_d = nc.dram_tensor("out", [T, H], F32, kind="ExternalOutput")

    with tile.TileContext(nc) as tc:
        with (
            tc.tile_pool(name="const", bufs=1) as const_pool,
            tc.tile_pool(name="x", bufs=1) as x_pool,
            tc.tile_pool(name="qk", bufs=1) as qk_pool,
            tc.tile_pool(name="vrows", bufs=1) as v_pool,
            tc.tile_pool(name="prows", bufs=1) as p_pool,
            tc.tile_pool(name="eps", bufs=3) as ep_pool,
            tc.tile_pool(name="projps", bufs=2, space="PSUM") as proj_ps,
            tc.tile_pool(name="sps", bufs=3, space="PSUM") as s_ps_pool,
            tc.tile_pool(name="accps", bufs=3, space="PSUM") as acc_ps,
        ):
            w_sb = {}
            for p in ("q", "k", "v"):
                w_sb[p] = const_pool.tile(
                    [128, ND, H], BF16, tag=f"w{p}", name=f"w{p}_sb"
                )
            bias_sb = const_pool.tile([128, 2], F32, tag="bias")
            mask_sb = const_pool.tile([128, 128], BF16, tag="mask")
            bvb_sb = const_pool.tile([128, 128], F32, tag="bvb")
            xh = {
                "b": [x_pool.tile([128, 1024], BF16, tag=f"xb{dc}", name=f"xb{dc}_sb")
                      for dc in range(ND)],
                "a": [x_pool.tile([128, 1024], BF16, tag=f"xa{dc}", name=f"xa{dc}_sb")
                      for dc in range(ND)],
            }

            # ---- input DMAs ----
            # SP queue: wk + all 16 x chunks (B half first, consumed first).
            # ACT queue: the small/early tensors, done before exp starts.
            nc.sync.dma_start(w_sb["k"][:], w_d["k"][:])
            nc.scalar.dma_start(w_sb["q"][:], w_d["q"][:])
            nc.scalar.dma_start(bias_sb[:], bias_d[:])
            for dc in range(ND):
                nc.sync.dma_start(xh["b"][dc][:], xb_d[dc, :, :])
            nc.scalar.dma_start(w_sb["v"][:], w_d["v"][:])
            nc.scalar.dma_start(bvb_sb[:], bvb_d[:])
            nc.scalar.dma_start(mask_sb[:], mask_d[:])
            for dc in range(ND):
                nc.sync.dma_start(xh["a"][dc][:], xa_d[dc, :, :])
            # pre-warm the ACT exp table while DMAs land
            warm = const_pool.tile([128, 1], F32, tag="warm")
            nc.scalar.activation(warm[:], bias_sb[:, 0:1], AF.Exp, scale=0.0)

            qk_sb = {"q": [None] * 4, "k": [None] * 4}
            v_rows = [None] * NT
            p_rows = [None] * NT

            def proj_pair(ncol):
                """k then q projection chunk for global cols
                [512*ncol, 512*ncol+512), dc-outer; returns psum tiles."""
                half = "b" if ncol >= 2 else "a"
                off = (ncol % 2) * 512
                ps = {}
                for p in ("k", "q"):
                    ps[p] = proj_ps.tile(
                        [128, 512], F32, tag="proj", name=f"ps_{p}{ncol}"
                    )
                for dc in range(ND):
                    for p in ("k", "q"):
                        nc.tensor.matmul(
                            ps[p][:],
                            w_sb[p][:, dc, :],
                            xh[half][dc][:, off : off + 512],
                            start=(dc == 0),
                            stop=(dc == ND - 1),
                        )
                return ps

            def proj_copy(ncol, ps):
                for i, p in enumerate(("q", "k")):
                    sb_t = qk_pool.tile(
                        [128, 512], BF16, tag=f"{p}{ncol}", name=f"{p}T{ncol}_sb"
                    )
                    nc.vector.tensor_scalar_add(
                        sb_t[:], ps[p][:], bias_sb[:, i : i + 1]
                    )
                    qk_sb[p][ncol] = sb_t

            def v_matmuls(si, vp):
                half = "b" if si >= 8 else "a"
                for dc in range(ND):
                    nc.tensor.matmul(
                        vp[:, 0:128],
                        xh[half][dc][:, (si % 8) * 128 : (si % 8 + 1) * 128],
                        w_sb["v"][:, dc, :],
                        start=(dc == 0),
                        stop=(dc == ND - 1),
                    )

            def v_finish(si, vp):
                vr = v_pool.tile([128, 129], BF16, tag=f"v{si}", name=f"v{si}_sb")
                nc.vector.tensor_add(vr[:, 0:128], vp[:, 0:128], bvb_sb[:])
                nc.vector.memset(vr[:, 128:129], 1.0)
                v_rows[si] = vr

            def v_row(si):
                vp = acc_ps.tile([128, 129], F32, tag="acc", name=f"v_ps{si}")
                v_matmuls(si, vp)
                v_finish(si, vp)

            def s_row(si):
                gc0 = si * 128
                pr = p_pool.tile(
                    [128, T - gc0], BF16, tag=f"p{si}", name=f"p{si}_sb"
                )
                c = gc0
                while c < T:
                    ce = min(T, (c // 512 + 1) * 512)
                    s_ps = s_ps_pool.tile(
                        [128, 512], F32, tag="sps", name=f"s_ps_{si}_{c}"
                    )
                    nc.tensor.matmul(
                        s_ps[:, 0 : ce - c],
                        qk_sb["q"][si // 4][:, (si % 4) * 128 : (si % 4 + 1) * 128],
                        qk_sb["k"][c // 512][:, c % 512 : c % 512 + (ce - c)],
                        start=True,
                        stop=True,
                    )
                    nc.scalar.activation(
                        pr[:, c - gc0 : ce - gc0],
                        s_ps[:, 0 : ce - c],
                        AF.Exp,
                        scale=SCALE,
                    )
                    c = ce
                # diagonal block: causal mask (keep s <= t)
                nc.vector.tensor_mul(pr[:, 0:128], pr[:, 0:128], mask_sb[:])
                p_rows[si] = pr

            # ---- group 1: {k3,q3} proj + v15..13, dc-outer, tracks DMA ----
            ps3 = {
                p: proj_ps.tile([128, 512], F32, tag="proj", name=f"ps_{p}3")
                for p in ("k", "q")
            }
            vp1 = {
                si: acc_ps.tile([128, 129], F32, tag="acc", name=f"v_ps{si}")
                for si in (15, 14, 13)
            }
            for dc in range(ND):
                for p in ("k", "q"):
                    nc.tensor.matmul(
                        ps3[p][:],
                        w_sb[p][:, dc, :],
                        xh["b"][dc][:, 512:1024],
                        start=(dc == 0),
                        stop=(dc == ND - 1),
                    )
                for si in (15, 14, 13):
                    nc.tensor.matmul(
                        vp1[si][:, 0:128],
                        xh["b"][dc][:, (si % 8) * 128 : (si % 8 + 1) * 128],
                        w_sb["v"][:, dc, :],
                        start=(dc == 0),
                        stop=(dc == ND - 1),
                    )
            proj_copy(3, ps3)
            for si in (15, 14, 13):
                v_finish(si, vp1[si])

            # ---- group 2: v12 + S rows 15..12 ----
            v_row(12)
            for si in (15, 14, 13, 12):
                s_row(si)

            # ---- remaining projections + rows, descending ----
            proj_copy(2, proj_pair(2))
            for si in (11, 10, 9, 8):
                v_row(si)
                s_row(si)
            proj_copy(1, proj_pair(1))
            for si in (7, 6, 5, 4):
                v_row(si)
                s_row(si)
            proj_copy(0, proj_pair(0))
            for si in (3, 2, 1, 0):
                v_row(si)
                s_row(si)

            # ---- O phase: per t-tile chains, si descending; t15 first
            # (longest chain chews ready rows while exps 0..3 finish),
            # t0 last (1 matmul -> shortest serial tail).
            for tj in range(NT - 1, -1, -1):
                o_ps = acc_ps.tile([128, 129], F32, tag="acc", name=f"o_ps{tj}")
                for si in range(tj, -1, -1):
                    nc.tensor.matmul(
                        o_ps[:],
                        p_rows[si][:, (tj - si) * 128 : (tj - si + 1) * 128],
                        v_rows[si][:],
                        start=(si == tj),
                        stop=(si == 0),
                    )
                recip = ep_pool.tile([128, 1], F32, tag="recip")
                nc.vector.reciprocal(recip[:], o_ps[:, 128:129])
                out_sb = ep_pool.tile([128, 128], F32, tag="outsb")
                nc.vector.tensor_scalar_mul(
                    out_sb[:], o_ps[:, 0:128], recip[:, 0:1]
                )
                nc.sync.dma_start(
                    out_d[tj * 128 : (tj + 1) * 128, :], out_sb[:]
                )

    nc.compile()
    return nc


_NC = None


def _get_nc():
    global _NC
    if _NC is None:
        _NC = build_nc()
    return _NC


def _make_in_maps(x, Wq, bq, Wk, bk, Wv, bv):
    bf = ml_dtypes.bfloat16

    def chunk_w(w):  # [1024, 128] -> [128, 8, 128] (partition, d-chunk, h)
        return np.ascontiguousarray(
            w.astype(bf).reshape(ND, 128, H).transpose(1, 0, 2)
        )

    shared = {
        "wq": chunk_w(Wq),
        "wk": chunk_w(Wk),
        "wv": chunk_w(Wv),
        "bias": np.ascontiguousarray(
            np.stack([bq, bk], axis=1).astype(np.float32)
        ),
        "mask": np.triu(np.ones((128, 128), dtype=np.float32)).astype(bf),
        "bvb": np.ascontiguousarray(
            np.broadcast_to(bv.astype(np.float32), (128, 128))
        ),
    }
    in_maps = []
    for i in range(B):
        m = dict(shared)
        xT = x[i].astype(bf).T  # [1024, 2048]
        xTc = xT.reshape(ND, 128, 2, 1024)  # [dc, part, half, 1024]
        m["xb"] = np.ascontiguousarray(xTc[:, :, 1, :])
        m["xa"] = np.ascontiguousarray(xTc[:, :, 0, :])
        in_maps.append(m)
    return in_maps


def _run(inputs, trace=False, **kw):
    nc = _get_nc()
    in_maps = _make_in_maps(**inputs)
    res = run_bass_kernel_spmd(nc, in_maps, core_ids=list(range(B)), trace=trace, **kw)
    out = np.stack([res.results[i]["out"] for i in range(B)], axis=0)
    return out.astype(np.float32), res


def kernel(x, Wq, bq, Wk, bk, Wv, bv):
    out, _ = _run(dict(x=x, Wq=Wq, bq=bq, Wk=Wk, bk=bk, Wv=Wv, bv=bv))
    return out


# revision 4
# speedup vs baseline: 1.0460x; 1.0460x over previous
"""Single-head causal attention on 8 TRN2 NeuronCores.

Problem: x[8, 2048, 1024] f32; Wq/Wk/Wv[1024, 128]; bq/bk/bv[128].
  q = x@Wq+bq; k = x@Wk+bk; v = x@Wv+bv
  scores[b,t,s] = k[b,t,:].q[b,s,:] / sqrt(128), causal (s<=t), softmax over s
  out = weights @ v   -> [8, 2048, 128] f32

Sharding: data-parallel over batch, one batch element per core. No collectives.

Per-core algorithm (T=2048, D=1024, H=128), matmuls in bf16. Design notes:
  - scores computed TRANSPOSED: S_T[s, t] = qT.T @ kT, so P_T = exp(S_T) is
    directly the stationary operand of out[t, 129] = P_T.T @ v_aug; the ones
    column of v_aug yields the softmax denominator for free.
  - bv is folded into the v rows (v' = v + bv): softmax weights sum to 1, so
    out/denom + bv == (P@(v+bv))/denom.  Kills the epilogue bias add.
  - input DMA triggers cost ~600-700ns of queue time each; they are split
    between the SP and ACT hardware DGE queues, ordered critical-first.
  - ONE set of PSUM pools lives for the whole kernel (no pool handoff
    barriers): proj ring (2 banks) + S ring (3) + v/O accumulator ring (3).
  - rows are processed in DESCENDING index order: S row si needs k cols
    >= 128*si, so high rows only need the last projection chunks; exp on ACT
    (the #2 engine) starts ~16us instead of ~31us and overlaps projections.
  - group 1 interleaves {k3,q3} proj with v rows 15..13 dc-outer so the PE
    tracks the xT DMA arrival rate chunk by chunk.
  - O accumulation chains run si DESCENDING (start=si==tj, stop=si==0): the
    last-exp'd rows (0..3) are needed at the END of each chain; the last O
    tile (t0) is the shortest chain, minimizing the serial tail.
"""

import math

import ml_dtypes
import numpy as np

import concourse.bass as bass
import concourse.mybir as mybir
import concourse.tile as tile
from concourse import bacc
from concourse.bass_utils import run_bass_kernel_spmd

B, T, D, H = 8, 2048, 1024, 128
NT = T // 128          # 16 t/s tiles
ND = D // 128          # 8 contraction chunks
SCALE = 1.0 / math.sqrt(H)

F32 = mybir.dt.float32
BF16 = mybir.dt.bfloat16
AF = mybir.ActivationFunctionType


def build_nc():
    nc = bacc.Bacc(
        "TRN2",
        target_bir_lowering=False,
        debug=False,
        num_devices=8,
    )

    # x[b].T split into 8 d-chunks x 2 column halves: B half = cols 1024:2048
    # (consumed first), A half = cols 0:1024. Each [128, 1024] contiguous.
    xb_d = nc.dram_tensor("xb", [ND, 128, 1024], BF16, kind="ExternalInput")
    xa_d = nc.dram_tensor("xa", [ND, 128, 1024], BF16, kind="ExternalInput")
    w_d = {
        p: nc.dram_tensor(f"w{p}", [128, ND, H], BF16, kind="ExternalInput")
        for p in ("q", "k", "v")
    }
    bias_d = nc.dram_tensor("bias", [H, 2], F32, kind="ExternalInput")
    mask_d = nc.dram_tensor("mask", [128, 128], BF16, kind="ExternalInput")
    bvb_d = nc.dram_tensor("bvb", [128, 128], F32, kind="ExternalInput")
    out_d = nc.dram_tensor("out", [T, H], F32, kind="ExternalOutput")

    with tile.TileContext(nc) as tc:
        with (
            tc.tile_pool(name="const", bufs=1) as const_pool,
            tc.tile_pool(name="x", bufs=1) as x_pool,
            tc.tile_pool(name="qk", bufs=1) as qk_pool,
            tc.tile_pool(name="vrows", bufs=1) as v_pool,
            tc.tile_pool(name="prows", bufs=1) as p_pool,
            tc.tile_pool(name="eps", bufs=3) as ep_pool,
            tc.tile_pool(name="projps", bufs=2, space="PSUM") as proj_ps,
            tc.tile_pool(name="sps", bufs=3, space="PSUM") as s_ps_pool,
            tc.tile_pool(name="accps", bufs=3, space="PSUM") as acc_ps,
        ):
            w_sb = {}
            for p in ("q", "k", "v"):
                w_sb[p] = const_pool.tile(
                    [128, ND, H], BF16, tag=f"w{p}", name=f"w{p}_sb"
                )
            bias_sb = const_pool.tile([128, 2], F32, tag="bias")
            mask_sb = const_pool.tile([128, 128], BF16, tag="mask")
            bvb_sb = const_pool.tile([128, 128], F32, tag="bvb")
            xh = {
                "b": [x_pool.tile([128, 1024], BF16, tag=f"xb{dc}", name=f"xb{dc}_sb")
                      for dc in range(ND)],
                "a": [x_pool.tile([128, 1024], BF16, tag=f"xa{dc}", name=f"xa{dc}_sb")
                      for dc in range(ND)],
            }

            # ---- input DMAs ----
            # SP queue: all 16 x chunks (B half first, consumed first).
            # ACT queue: weights + small tensors, done before exp starts.
            nc.scalar.dma_start(w_sb["k"][:], w_d["k"][:])
            nc.scalar.dma_start(w_sb["q"][:], w_d["q"][:])
            nc.scalar.dma_start(w_sb["v"][:], w_d["v"][:])
            nc.scalar.dma_start(bias_sb[:], bias_d[:])
            nc.scalar.dma_start(bvb_sb[:], bvb_d[:])
            nc.scalar.dma_start(mask_sb[:], mask_d[:])
            for dc in range(ND):
                nc.sync.dma_start(xh["b"][dc][:], xb_d[dc, :, :])
            for dc in range(ND):
                nc.sync.dma_start(xh["a"][dc][:], xa_d[dc, :, :])
            # pre-warm the ACT exp table while DMAs land
            warm = const_pool.tile([128, 1], F32, tag="warm")
            nc.scalar.activation(warm[:], bias_sb[:, 0:1], AF.Exp, scale=0.0)

            qk_sb = {"q": [None] * 4, "k": [None] * 4}
            v_rows = [None] * NT
            p_rows = [None] * NT

            def proj_pair(ncol):
                """k then q projection chunk for global cols
                [512*ncol, 512*ncol+512), dc-outer; returns psum tiles."""
                half = "b" if ncol >= 2 else "a"
                off = (ncol % 2) * 512
                ps = {}
                for p in ("k", "q"):
                    ps[p] = proj_ps.tile(
                        [128, 512], F32, tag="proj", name=f"ps_{p}{ncol}"
                    )
                for dc in range(ND):
                    for p in ("k", "q"):
                        nc.tensor.matmul(
                            ps[p][:],
                            w_sb[p][:, dc, :],
                            xh[half][dc][:, off : off + 512],
                            start=(dc == 0),
                            stop=(dc == ND - 1),
                        )
                return ps

            def proj_copy(ncol, ps):
                for i, p in enumerate(("q", "k")):
                    sb_t = qk_pool.tile(
                        [128, 512], BF16, tag=f"{p}{ncol}", name=f"{p}T{ncol}_sb"
                    )
                    nc.vector.tensor_scalar_add(
                        sb_t[:], ps[p][:], bias_sb[:, i : i + 1]
                    )
                    qk_sb[p][ncol] = sb_t

            def v_matmuls(si, vp):
                half = "b" if si >= 8 else "a"
                for dc in range(ND):
                    nc.tensor.matmul(
                        vp[:, 0:128],
                        xh[half][dc][:, (si % 8) * 128 : (si % 8 + 1) * 128],
                        w_sb["v"][:, dc, :],
                        start=(dc == 0),
                        stop=(dc == ND - 1),
                    )

            def v_finish(si, vp):
                vr = v_pool.tile([128, 129], BF16, tag=f"v{si}", name=f"v{si}_sb")
                nc.vector.tensor_add(vr[:, 0:128], vp[:, 0:128], bvb_sb[:])
                nc.vector.memset(vr[:, 128:129], 1.0)
                v_rows[si] = vr

            def v_row(si):
                vp = acc_ps.tile([128, 129], F32, tag="acc", name=f"v_ps{si}")
                v_matmuls(si, vp)
                v_finish(si, vp)

            def s_row(si):
                gc0 = si * 128
                pr = p_pool.tile(
                    [128, T - gc0], BF16, tag=f"p{si}", name=f"p{si}_sb"
                )
                c = gc0
                while c < T:
                    ce = min(T, (c // 512 + 1) * 512)
                    s_ps = s_ps_pool.tile(
                        [128, 512], F32, tag="sps", name=f"s_ps_{si}_{c}"
                    )
                    nc.tensor.matmul(
                        s_ps[:, 0 : ce - c],
                        qk_sb["q"][si // 4][:, (si % 4) * 128 : (si % 4 + 1) * 128],
                        qk_sb["k"][c // 512][:, c % 512 : c % 512 + (ce - c)],
                        start=True,
                        stop=True,
                    )
                    nc.scalar.activation(
                        pr[:, c - gc0 : ce - gc0],
                        s_ps[:, 0 : ce - c],
                        AF.Exp,
                        scale=SCALE,
                    )
                    c = ce
                # diagonal block: causal mask (keep s <= t)
                nc.vector.tensor_mul(pr[:, 0:128], pr[:, 0:128], mask_sb[:])
                p_rows[si] = pr

            # ---- group 1: {k3,q3} proj + v15..13, dc-outer, tracks DMA ----
            ps3 = {
                p: proj_ps.tile([128, 512], F32, tag="proj", name=f"ps_{p}3")
                for p in ("k", "q")
            }
            vp1 = {
                si: acc_ps.tile([128, 129], F32, tag="acc", name=f"v_ps{si}")
                for si in (15, 14, 13)
            }
            for dc in range(ND):
                for p in ("k", "q"):
                    nc.tensor.matmul(
                        ps3[p][:],
                        w_sb[p][:, dc, :],
                        xh["b"][dc][:, 512:1024],
                        start=(dc == 0),
                        stop=(dc == ND - 1),
                    )
                for si in (15, 14, 13):
                    nc.tensor.matmul(
                        vp1[si][:, 0:128],
                        xh["b"][dc][:, (si % 8) * 128 : (si % 8 + 1) * 128],
                        w_sb["v"][:, dc, :],
                        start=(dc == 0),
                        stop=(dc == ND - 1),
                    )
            proj_copy(3, ps3)
            for si in (15, 14, 13):
                v_finish(si, vp1[si])

            # ---- group 2: v12 + S rows 15..12 ----
            v_row(12)
            for si in (15, 14, 13, 12):
                s_row(si)

            # ---- remaining projections + rows, descending ----
            proj_copy(2, proj_pair(2))
            for si in (11, 10, 9, 8):
                v_row(si)
                s_row(si)
            proj_copy(1, proj_pair(1))
            for si in (7, 6, 5, 4):
                v_row(si)
                s_row(si)
            proj_copy(0, proj_pair(0))
            # rows 3..0 have the longest rows (most exp work on ACT); while
            # ACT chews them, pre-run O t15's si 15..4 accumulation to keep
            # the PE fed (the chain pauses, holding its PSUM slot).
            o15 = acc_ps.tile([128, 129], F32, tag="acc", name="o_ps15")

            def o_chain(tj, o_ps, si_hi, si_lo):
                for si in range(si_hi, si_lo - 1, -1):
                    nc.tensor.matmul(
                        o_ps[:],
                        p_rows[si][:, (tj - si) * 128 : (tj - si + 1) * 128],
                        v_rows[si][:],
                        start=(si == tj),
                        stop=(si == 0),
                    )

            def epilogue(tj, o_ps):
                recip = ep_pool.tile([128, 1], F32, tag="recip", bufs=4)
                nc.vector.reciprocal(recip[:], o_ps[:, 128:129])
                out_sb = ep_pool.tile([128, 128], F32, tag="outsb", bufs=8)
                if tj % 2 == 0:
                    nc.scalar.activation(
                        out_sb[:], o_ps[:, 0:128], AF.Identity,
                        scale=recip[:, 0:1],
                    )
                else:
                    nc.vector.tensor_scalar_mul(
                        out_sb[:], o_ps[:, 0:128], recip[:, 0:1]
                    )
                dma_eng = nc.gpsimd if tj % 2 == 0 and tj >= 4 else nc.sync
                dma_eng.dma_start(
                    out_d[tj * 128 : (tj + 1) * 128, :], out_sb[:]
                )

            v_row(3)
            s_row(3)
            o_chain(15, o15, 15, 4)
            for si in (2, 1, 0):
                v_row(si)
                s_row(si)
            o_chain(15, o15, 3, 0)
            epilogue(15, o15)

            # ---- remaining O tiles: per t-tile chains, si descending;
            # t0 last (1 matmul -> shortest serial tail).
            for tj in range(NT - 2, -1, -1):
                o_ps = acc_ps.tile([128, 129], F32, tag="acc", name=f"o_ps{tj}")
                o_chain(tj, o_ps, tj, 0)
                epilogue(tj, o_ps)

    nc.compile()
    return nc


_NC = None


def _get_nc():
    global _NC
    if _NC is None:
        _NC = build_nc()
    return _NC


def _make_in_maps(x, Wq, bq, Wk, bk, Wv, bv):
    bf = ml_dtypes.bfloat16

    def chunk_w(w):  # [1024, 128] -> [128, 8, 128] (partition, d-chunk, h)
        return np.ascontiguousarray(
            w.astype(bf).reshape(ND, 128, H).transpose(1, 0, 2)
        )

    shared = {
        "wq": chunk_w(Wq),
        "wk": chunk_w(Wk),
        "wv": chunk_w(Wv),
        "bias": np.ascontiguousarray(
            np.stack([bq, bk], axis=1).astype(np.float32)
        ),
        "mask": np.triu(np.ones((128, 128), dtype=np.float32)).astype(bf),
        "bvb": np.ascontiguousarray(
            np.broadcast_to(bv.astype(np.float32), (128, 128))
        ),
    }
    in_maps = []
    for i in range(B):
        m = dict(shared)
        xT = x[i].astype(bf).T  # [1024, 2048]
        xTc = xT.reshape(ND, 128, 2, 1024)  # [dc, part, half, 1024]
        m["xb"] = np.ascontiguousarray(xTc[:, :, 1, :])
        m["xa"] = np.ascontiguousarray(xTc[:, :, 0, :])
        in_maps.append(m)
    return in_maps


def _run(inputs, trace=False, **kw):
    nc = _get_nc()
    in_maps = _make_in_maps(**inputs)
    res = run_bass_kernel_spmd(nc, in_maps, core_ids=list(range(B)), trace=trace, **kw)
    out = np.stack([res.results[i]["out"] for i in range(B)], axis=0)
    return out.astype(np.float32), res


def kernel(x, Wq, bq, Wk, bk, Wv, bv):
    out, _ = _run(dict(x=x, Wq=Wq, bq=bq, Wk=Wk, bk=bk, Wv=Wv, bv=bv))
    return out
